# revision 1
# baseline (speedup 1.0000x reference)
"""Trainium2 Bass kernel for nn_Cont_InfoNCE (pairwise max cross-correlation + CE loss).

Math: the reference's irfft(F1[i] * conj(F2[j]) / power) is the linear
cross-correlation of the centered rows at every lag, scaled by the positive
constant 1/(power*(T-1)).  max over lags therefore commutes with the scaling,
so dist[i,j] = max_l sum_t f1c[i,t] * f2c[j,t+l] / (1023*s1[i]*s2[j]).

We compute the correlation at all lags as dense fp8e4m3 DoubleRow matmuls on
the tensor engine (fp32 PSUM accumulation; fp8 rounding contributes ~1e-5
relative loss error), max-reduce over lags on the vector engine, and do the
row-wise CE on device.  Sharding: rows of zis across the 8 cores (32 rows
each), zjs replicated; each core emits a partial loss scalar and the host
sums the 8 partials.

Tiling (per core; A = centered local zis rows (32,1024), B = centered zjs):
  Apad[i]    = [0^255, A[i], 0^257]                       (32, 1536) fp8
  Tau[t,i,u] = Apad[i, u+t]          (Hankel gather via DMA from DRAM)
  BT[t,c,j]  = B[j, 128c+t]          (PE transposes, bf16 -> fp8 on copy-out)
  for lam in 0..15, jt in 0..1, ic in 0..7:
    psum[j,ii,d'] += BT[:, 2dc:2dc+2, jtile].T @ Tau[:, ic, u0:u0+256]  (DoubleRow)
      over dc with u0 = 128*(2dc - lam + 9); pair halves are the two
      128-chunks of t, matching the production [P, ksub, free] convention.
  psum[j,ii,d'] equals C[i, j, l] at lag l = 128*lam - 897 - d', covering
  every lag in [-1024, 1023] exactly once (the l = -1024 slot is identically
  0, mirroring the reference's zero-overlap k=1024 slot).
"""

import sys

if "/opt/trn_rl_repo" not in sys.path:
    sys.path.insert(0, "/opt/trn_rl_repo")

from contextlib import ExitStack

import numpy as np

import concourse.bass as bass
import concourse.mybir as mybir
from concourse import bacc, tile
from concourse.bass_utils import run_bass_kernel_spmd
from concourse.masks import make_identity

F32 = mybir.dt.float32
BF16 = mybir.dt.bfloat16
FP8 = mybir.dt.float8e4
I32 = mybir.dt.int32
X = mybir.AxisListType.X
ALU = mybir.AluOpType
ACT = mybir.ActivationFunctionType
DROW = mybir.MatmulPerfMode.DoubleRow

M, T = 256, 1024
NCORES = 8
NLOC = M // NCORES  # 32 rows of zis per core
NIC = 4             # i-rows per i-chunk
NCHUNK = NLOC // NIC  # 8 i-chunks
TAU_U = 1408        # Hankel window extent: covers e0 in [-1, 8], +256 window
APAD = 1536         # 255 zeros + 1024 + 257 zeros


def _rsqrt_scaled(nc, pool, out, ss, k, parts, tag):
    """out = sqrt(k / ss), elementwise on a (parts,1) fp32 column.

    vector.reciprocal (accurate iterative divide) + ACT Sqrt + one Newton
    step to wash out the Sqrt table's loose ULP budget.
    """
    a = pool.tile([parts, 1], F32, tag=tag + "_a")
    nc.vector.reciprocal(a, ss)
    v = pool.tile([parts, 1], F32, tag=tag + "_v")
    nc.vector.tensor_scalar_mul(v, a, float(k))
    y0 = pool.tile([parts, 1], F32, tag=tag + "_y0")
    nc.scalar.sqrt(y0, v)
    ry = pool.tile([parts, 1], F32, tag=tag + "_ry")
    nc.vector.reciprocal(ry, y0)
    t2 = pool.tile([parts, 1], F32, tag=tag + "_t2")
    # t2 = (v * 0.5) * (1/y0)
    nc.vector.scalar_tensor_tensor(t2, in0=v, scalar=0.5, in1=ry, op0=ALU.mult, op1=ALU.mult)
    # out = (y0 * 0.5) + t2
    nc.vector.scalar_tensor_tensor(out, in0=y0, scalar=0.5, in1=t2, op0=ALU.mult, op1=ALU.add)


def _row_stats(nc, pool, in_tile, parts, tag):
    """Returns (negmean, ss) for each row of in_tile, computed on ScalarE.

    ss = sum((x - mean)^2) = sum(x^2) - T*mean^2; the only DVE use is the
    final tiny (parts,1) combine.
    """
    junk1 = pool.tile([parts, T], BF16, tag=tag + "_j1")
    rsum = pool.tile([parts, 1], F32, tag=tag + "_rsum")
    nc.scalar.activation(junk1, in_tile, ACT.Identity, accum_out=rsum)
    junk2 = pool.tile([parts, T], BF16, tag=tag + "_j2")
    ssraw = pool.tile([parts, 1], F32, tag=tag + "_ssraw")
    nc.scalar.activation(junk2, in_tile, ACT.Square, accum_out=ssraw)
    negmean = pool.tile([parts, 1], F32, tag=tag + "_negmean")
    nc.scalar.mul(negmean, rsum, -1.0 / T)
    mu2 = pool.tile([parts, 1], F32, tag=tag + "_mu2")
    nc.scalar.activation(mu2, negmean, ACT.Square)
    ss = pool.tile([parts, 1], F32, tag=tag + "_ss")
    nc.vector.scalar_tensor_tensor(ss, in0=mu2, scalar=-float(T), in1=ssraw, op0=ALU.mult, op1=ALU.add)
    return negmean, ss


def build_nc():
    nc = bacc.Bacc("TRN2", target_bir_lowering=False)
    zis_loc = nc.dram_tensor("zis_loc", [NLOC, T], F32, kind="ExternalInput")
    zjs_full = nc.dram_tensor("zjs_full", [M, T], F32, kind="ExternalInput")
    speeds_loc = nc.dram_tensor("speeds_loc", [NLOC, 1], I32, kind="ExternalInput")
    loss_part = nc.dram_tensor("loss_part", [1, 1], F32, kind="ExternalOutput")

    with tile.TileContext(nc) as tc, ExitStack() as ctx:
        consts = ctx.enter_context(tc.tile_pool(name="consts", bufs=1))
        prep = ctx.enter_context(tc.tile_pool(name="prep", bufs=2))
        dram = ctx.enter_context(tc.tile_pool(name="dram", bufs=1, space="DRAM"))
        taup = ctx.enter_context(tc.tile_pool(name="taup", bufs=3))
        ps_aux = ctx.enter_context(tc.tile_pool(name="ps_aux", bufs=2, space="PSUM"))
        ps_main = ctx.enter_context(tc.tile_pool(name="ps_main", bufs=3, space="PSUM"))

        # ---------------- constants ----------------
        ident_bf = consts.tile([128, 128], BF16)
        make_identity(nc, ident_bf)
        ident_f32 = consts.tile([128, 128], F32)
        make_identity(nc, ident_f32)
        ones_col = consts.tile([NLOC, 1], F32)
        nc.gpsimd.memset(ones_col, 1.0)
        jidx_i = consts.tile([NLOC, M], I32)
        nc.gpsimd.iota(jidx_i, [[1, M]], base=0, channel_multiplier=0)
        jidx_f = consts.tile([NLOC, M], F32)
        nc.scalar.copy(jidx_f, jidx_i)
        sp_i = prep.tile([NLOC, 1], I32)
        nc.sync.dma_start(sp_i, speeds_loc[:, :])
        sp_f = prep.tile([NLOC, 1], F32)
        nc.scalar.copy(sp_f, sp_i)

        # ---------------- A (local zis rows): stats, center -> fp8 Apad ------
        a_in = prep.tile([NLOC, T], F32)
        nc.sync.dma_start(a_in, zis_loc[:, :])
        nega, ss1 = _row_stats(nc, prep, a_in, NLOC, "a")
        r1 = prep.tile([NLOC, 1], F32)
        _rsqrt_scaled(nc, prep, r1, ss1, 1.0 / (T - 1), NLOC, "r1")  # 1/((T-1)*s1)

        apad_sb = prep.tile([NLOC, APAD], FP8)
        nc.gpsimd.memset(apad_sb, 0.0)
        nc.scalar.activation(apad_sb[:, 255:255 + T], a_in, ACT.Identity, bias=nega)
        apad_d = dram.tile([NLOC, APAD], FP8)
        nc.sync.dma_start(apad_d[:, :], apad_sb[:, :])

        # ---------------- B (all zjs rows): stats, center -> bf16 ------------
        bc_tiles = []
        r2_tiles = []
        for jt in range(2):
            b_in = prep.tile([128, T], F32, tag="b_in")
            nc.sync.dma_start(b_in, zjs_full[jt * 128:(jt + 1) * 128, :])
            negb, ss2 = _row_stats(nc, prep, b_in, 128, "b")
            r2 = consts.tile([128, 1], F32, tag=f"r2_{jt}", name=f"r2_{jt}")
            _rsqrt_scaled(nc, prep, r2, ss2, float(T - 1), 128, "r2")  # 1/s2
            r2_tiles.append(r2)
            bc = consts.tile([128, T], BF16, tag=f"bc_{jt}", name=f"bc_{jt}")
            nc.scalar.activation(bc, b_in, ACT.Identity, bias=negb)
            bc_tiles.append(bc)

        # -------- BT[t, c, j] = B[j, 128c+t] via PE transposes, fp8 ----------
        bt8 = consts.tile([128, 8, M], FP8)
        for jt in range(2):
            for c in range(8):
                ps_t = ps_aux.tile([128, 128], BF16, tag="aux")
                nc.tensor.transpose(ps_t, bc_tiles[jt][:, 128 * c:128 * (c + 1)], ident_bf)
                nc.scalar.copy(bt8[:, c, jt * 128:(jt + 1) * 128], ps_t)

        # ---------------- main correlation loop ------------------------------
        cmax_p = [
            consts.tile([128, 16, NLOC], F32, tag=f"cmax_{jt}", name=f"cmax_{jt}")
            for jt in range(2)
        ]
        for ic in range(NCHUNK):
            tau = taup.tile([128, NIC, TAU_U], FP8, tag="tau")
            src = apad_d[NIC * ic:NIC * (ic + 1), 0:TAU_U]
            v = src.unsqueeze(0).broadcast_to((128, NIC, TAU_U))
            lst = v.ap
            lst[0] = [1, 128]  # Hankel: dest partition t reads Apad at +t elements
            v.ap = lst
            nc.sync.dma_start(tau[:, :, :], v)
            for jt in range(2):
                for lp in range(8):  # lambda pairs -> one 2-bank psum tile
                    ps = ps_main.tile([128, 2, NIC, 128], F32, tag="grp")
                    for q in range(2):
                        lam = 2 * lp + q
                        # valid double-chunks: e0 = 2dc - lam + 8 in [-1, 8]
                        dcs = [dc for dc in range(4) if -1 <= 2 * dc - lam + 8 <= 8]
                        for k, dc in enumerate(dcs):
                            u0 = 128 * (2 * dc - lam + 9)
                            rhs = tau[:, :, u0:u0 + 256].rearrange(
                                "p r (i d) -> p i r d", i=2
                            )
                            nc.tensor.matmul(
                                ps[:, q],
                                lhsT=bt8[:, 2 * dc:2 * dc + 2, jt * 128:(jt + 1) * 128],
                                rhs=rhs,
                                perf_mode=DROW,
                                start=(k == 0),
                                stop=(k == len(dcs) - 1),
                            )
                    nc.vector.reduce_max(
                        cmax_p[jt][:, 2 * lp:2 * lp + 2, NIC * ic:NIC * (ic + 1)],
                        ps[:, :, :, :],
                        axis=X,
                    )

        # ---------------- normalize + transpose to (i, j) ---------------------
        dist_t = prep.tile([NLOC, M], F32)
        for jt in range(2):
            cm2 = prep.tile([128, NLOC], F32, tag="cm2")
            nc.vector.reduce_max(cm2, cmax_p[jt].rearrange("p l i -> p i l"), axis=X)
            cms = prep.tile([128, NLOC], F32, tag="cms")
            nc.vector.tensor_scalar(cms, cm2, r2_tiles[jt], None, op0=ALU.mult)
            ps_d = ps_aux.tile([NLOC, 128], F32, tag="aux")
            nc.tensor.transpose(ps_d, cms, ident_f32)
            nc.vector.tensor_scalar(dist_t[:, jt * 128:(jt + 1) * 128], ps_d, r1, None, op0=ALU.mult)

        # ---------------- cross-entropy (sum over local rows) -----------------
        mrow = prep.tile([NLOC, 1], F32)
        nc.vector.reduce_max(mrow, dist_t, axis=X)
        negm = prep.tile([NLOC, 1], F32)
        nc.vector.tensor_scalar_mul(negm, mrow, -1.0)
        expj = prep.tile([NLOC, M], F32)
        sumexp = prep.tile([NLOC, 1], F32)
        nc.scalar.activation(expj, dist_t, ACT.Exp, bias=negm, accum_out=sumexp)
        lse = prep.tile([NLOC, 1], F32)
        nc.scalar.activation(lse, sumexp, ACT.Ln)
        onehot = prep.tile([NLOC, M], F32)
        nc.vector.tensor_scalar(onehot, jidx_f, sp_f, None, op0=ALU.is_equal)
        junk_p = prep.tile([NLOC, M], F32)
        picked = prep.tile([NLOC, 1], F32)
        nc.vector.scalar_tensor_tensor(
            junk_p, in0=dist_t, scalar=1.0, in1=onehot, op0=ALU.mult, op1=ALU.mult, accum_out=picked
        )
        term = prep.tile([NLOC, 1], F32)
        nc.vector.tensor_add(term, lse, mrow)
        term2 = prep.tile([NLOC, 1], F32)
        nc.vector.tensor_sub(term2, term, picked)
        ps_l = ps_aux.tile([1, 1], F32, tag="aux")
        nc.tensor.matmul(ps_l, lhsT=term2, rhs=ones_col, start=True, stop=True)
        lsb = prep.tile([1, 1], F32)
        nc.vector.tensor_copy(lsb, ps_l)
        nc.sync.dma_start(loss_part[:, :], lsb)

    nc.finalize()
    return nc


_NC_CACHE = None
LAST_RESULT = None


def run(zis, zjs, speeds, trace=False):
    global _NC_CACHE, LAST_RESULT
    if _NC_CACHE is None:
        _NC_CACHE = build_nc()
    zis = np.ascontiguousarray(np.asarray(zis), dtype=np.float32)
    zjs = np.ascontiguousarray(np.asarray(zjs), dtype=np.float32)
    sp = np.asarray(speeds).astype(np.int32).reshape(M, 1)
    in_maps = [
        {
            "zis_loc": np.ascontiguousarray(zis[c * NLOC:(c + 1) * NLOC]),
            "zjs_full": zjs,
            "speeds_loc": np.ascontiguousarray(sp[c * NLOC:(c + 1) * NLOC]),
        }
        for c in range(NCORES)
    ]
    res = run_bass_kernel_spmd(_NC_CACHE, in_maps, core_ids=list(range(NCORES)), trace=trace)
    LAST_RESULT = res
    total = sum(float(r["loss_part"][0, 0]) for r in res.results)
    return np.float32(total)


def kernel(zis, zjs, speeds):
    return run(zis, zjs, speeds, trace=False)



# revision 2
# speedup vs baseline: 10.6700x; 10.6700x over previous
"""Trainium2 Bass kernel for nn_Cont_InfoNCE (pairwise max cross-correlation + CE loss).

Math: the reference's irfft(F1[i] * conj(F2[j]) / power) is the linear
cross-correlation of the centered rows at every lag, scaled by the positive
constant 1/(power*(T-1)).  max over lags therefore commutes with the scaling,
so dist[i,j] = max_l sum_t f1c[i,t] * f2c[j,t+l] / (1023*s1[i]*s2[j]).

We compute the correlation at all lags as dense fp8e4m3 DoubleRow matmuls on
the tensor engine (fp32 PSUM accumulation; fp8 rounding contributes ~1e-5
relative loss error), max-reduce over lags on the vector engine, and do the
row-wise CE on device.

Sharding + host I/O (latency-optimized for the axon-tunneled cores: the
tunnel has ~75 ms RTT and ~80 MB/s bandwidth, so wire bytes and round
trips dominate; the NEFF itself is sub-ms):
  - zis and zjs ship as fp8e4m3 (quantizing the *inputs* to fp8 moves the
    loss by ~3e-6 relative — measured against the f32 reference — because
    the kernel's matmuls are fp8 anyway and the CE is insensitive).
  - rows of both zis and zjs are sharded across the 8 cores (32 rows
    each); the kernel AllGathers zjs on-device over NeuronLink instead of
    the host shipping 8 replicas through the tunnel.
  - each core's partial CE loss is AllReduced on-device; the host fetches
    a single replicated (1,1) scalar from one shard.
  - the jitted shard_map executable is built once and cached; calling
    bass_utils.run_bass_kernel_spmd per-call would rebuild jax.jit each
    time (~330 ms of retrace/relower per call).

Tiling (per core; A = centered local zis rows (32,1024), B = centered zjs):
  Apad[i]    = [0^255, A[i], 0^257]                       (32, 1536) fp8
  Tau[t,i,u] = Apad[i, u+t]          (Hankel gather via DMA from DRAM)
  BT[t,c,j]  = B[j, 128c+t]          (PE transposes, bf16 -> fp8 on copy-out)
  for lam in 0..15, jt in 0..1, ic in 0..7:
    psum[j,ii,d'] += BT[:, 2dc:2dc+2, jtile].T @ Tau[:, ic, u0:u0+256]  (DoubleRow)
      over dc with u0 = 128*(2dc - lam + 9); pair halves are the two
      128-chunks of t, matching the production [P, ksub, free] convention.
  psum[j,ii,d'] equals C[i, j, l] at lag l = 128*lam - 897 - d', covering
  every lag in [-1024, 1023] exactly once (the l = -1024 slot is identically
  0, mirroring the reference's zero-overlap k=1024 slot).
"""

import sys

if "/opt/trn_rl_repo" not in sys.path:
    sys.path.insert(0, "/opt/trn_rl_repo")

from contextlib import ExitStack

import numpy as np

import concourse.bass as bass
import concourse.mybir as mybir
from concourse import bacc, tile
from concourse.masks import make_identity

F32 = mybir.dt.float32
BF16 = mybir.dt.bfloat16
FP8 = mybir.dt.float8e4
I32 = mybir.dt.int32
X = mybir.AxisListType.X
ALU = mybir.AluOpType
ACT = mybir.ActivationFunctionType
DROW = mybir.MatmulPerfMode.DoubleRow

M, T = 256, 1024
NCORES = 8
NLOC = M // NCORES  # 32 rows of zis (and of zjs) per core
NIC = 4             # i-rows per i-chunk
NCHUNK = NLOC // NIC  # 8 i-chunks
TAU_U = 1408        # Hankel window extent: covers e0 in [-1, 8], +256 window
APAD = 1536         # 255 zeros + 1024 + 257 zeros
GROUPS = [list(range(NCORES))]

NP_FP8 = mybir.dt.np(FP8)


def _rsqrt_scaled(nc, pool, out, ss, k, parts, tag):
    """out = sqrt(k / ss), elementwise on a (parts,1) fp32 column.

    vector.reciprocal (accurate iterative divide) + ACT Sqrt + one Newton
    step to wash out the Sqrt table's loose ULP budget.
    """
    a = pool.tile([parts, 1], F32, tag=tag + "_a")
    nc.vector.reciprocal(a, ss)
    v = pool.tile([parts, 1], F32, tag=tag + "_v")
    nc.vector.tensor_scalar_mul(v, a, float(k))
    y0 = pool.tile([parts, 1], F32, tag=tag + "_y0")
    nc.scalar.sqrt(y0, v)
    ry = pool.tile([parts, 1], F32, tag=tag + "_ry")
    nc.vector.reciprocal(ry, y0)
    t2 = pool.tile([parts, 1], F32, tag=tag + "_t2")
    # t2 = (v * 0.5) * (1/y0)
    nc.vector.scalar_tensor_tensor(t2, in0=v, scalar=0.5, in1=ry, op0=ALU.mult, op1=ALU.mult)
    # out = (y0 * 0.5) + t2
    nc.vector.scalar_tensor_tensor(out, in0=y0, scalar=0.5, in1=t2, op0=ALU.mult, op1=ALU.add)


def _row_stats(nc, pool, in_tile, parts, tag):
    """Returns (negmean, ss) for each row of in_tile, computed on ScalarE.

    ss = sum((x - mean)^2) = sum(x^2) - T*mean^2; the only DVE use is the
    final tiny (parts,1) combine.
    """
    junk1 = pool.tile([parts, T], BF16, tag=tag + "_j1")
    rsum = pool.tile([parts, 1], F32, tag=tag + "_rsum")
    nc.scalar.activation(junk1, in_tile, ACT.Identity, accum_out=rsum)
    junk2 = pool.tile([parts, T], BF16, tag=tag + "_j2")
    ssraw = pool.tile([parts, 1], F32, tag=tag + "_ssraw")
    nc.scalar.activation(junk2, in_tile, ACT.Square, accum_out=ssraw)
    negmean = pool.tile([parts, 1], F32, tag=tag + "_negmean")
    nc.scalar.mul(negmean, rsum, -1.0 / T)
    mu2 = pool.tile([parts, 1], F32, tag=tag + "_mu2")
    nc.scalar.activation(mu2, negmean, ACT.Square)
    ss = pool.tile([parts, 1], F32, tag=tag + "_ss")
    nc.vector.scalar_tensor_tensor(ss, in0=mu2, scalar=-float(T), in1=ssraw, op0=ALU.mult, op1=ALU.add)
    return negmean, ss


def build_nc():
    nc = bacc.Bacc("TRN2", target_bir_lowering=False, num_devices=NCORES)
    zis_loc = nc.dram_tensor("zis_loc", [NLOC, T], FP8, kind="ExternalInput")
    zjs_shard = nc.dram_tensor("zjs_shard", [NLOC, T], FP8, kind="ExternalInput")
    speeds_loc = nc.dram_tensor("speeds_loc", [NLOC, 1], I32, kind="ExternalInput")
    loss_part = nc.dram_tensor("loss_part", [1, 1], F32, kind="ExternalOutput")

    with tile.TileContext(nc) as tc, ExitStack() as ctx:
        consts = ctx.enter_context(tc.tile_pool(name="consts", bufs=1))
        prep = ctx.enter_context(tc.tile_pool(name="prep", bufs=2))
        dram = ctx.enter_context(tc.tile_pool(name="dram", bufs=1, space="DRAM"))
        taup = ctx.enter_context(tc.tile_pool(name="taup", bufs=3))
        ps_aux = ctx.enter_context(tc.tile_pool(name="ps_aux", bufs=2, space="PSUM"))
        ps_main = ctx.enter_context(tc.tile_pool(name="ps_main", bufs=3, space="PSUM"))

        # ------------- zjs AllGather (fp8 shard -> full 256 rows) ------------
        zj_in = dram.tile([NLOC, T], FP8, tag="zj_in", name="zj_in")
        nc.gpsimd.dma_start(zj_in[:, :], zjs_shard[:, :])
        zj_all = dram.tile([M, T], FP8, tag="zj_all", name="zj_all")
        nc.gpsimd.collective_compute(
            "AllGather",
            mybir.AluOpType.bypass,
            replica_groups=GROUPS,
            ins=[zj_in.opt()],
            outs=[zj_all.opt()],
        )

        # ---------------- constants ----------------
        ident_bf = consts.tile([128, 128], BF16)
        make_identity(nc, ident_bf)
        ident_f32 = consts.tile([128, 128], F32)
        make_identity(nc, ident_f32)
        ones_col = consts.tile([NLOC, 1], F32)
        nc.gpsimd.memset(ones_col, 1.0)
        jidx_i = consts.tile([NLOC, M], I32)
        nc.gpsimd.iota(jidx_i, [[1, M]], base=0, channel_multiplier=0)
        jidx_f = consts.tile([NLOC, M], F32)
        nc.scalar.copy(jidx_f, jidx_i)
        sp_i = prep.tile([NLOC, 1], I32)
        nc.sync.dma_start(sp_i, speeds_loc[:, :])
        sp_f = prep.tile([NLOC, 1], F32)
        nc.scalar.copy(sp_f, sp_i)

        # ---------------- A (local zis rows): stats, center -> fp8 Apad ------
        a_in = prep.tile([NLOC, T], FP8)
        nc.sync.dma_start(a_in, zis_loc[:, :])
        nega, ss1 = _row_stats(nc, prep, a_in, NLOC, "a")
        r1 = prep.tile([NLOC, 1], F32)
        _rsqrt_scaled(nc, prep, r1, ss1, 1.0 / (T - 1), NLOC, "r1")  # 1/((T-1)*s1)

        apad_sb = prep.tile([NLOC, APAD], FP8)
        nc.gpsimd.memset(apad_sb, 0.0)
        nc.scalar.activation(apad_sb[:, 255:255 + T], a_in, ACT.Identity, bias=nega)
        apad_d = dram.tile([NLOC, APAD], FP8, tag="apad", name="apad")
        nc.sync.dma_start(apad_d[:, :], apad_sb[:, :])

        # ---------------- B (all zjs rows): stats, center -> bf16 ------------
        bc_tiles = []
        r2_tiles = []
        for jt in range(2):
            b_in = prep.tile([128, T], FP8, tag="b_in")
            nc.sync.dma_start(b_in, zj_all[jt * 128:(jt + 1) * 128, :])
            negb, ss2 = _row_stats(nc, prep, b_in, 128, "b")
            r2 = consts.tile([128, 1], F32, tag=f"r2_{jt}", name=f"r2_{jt}")
            _rsqrt_scaled(nc, prep, r2, ss2, float(T - 1), 128, "r2")  # 1/s2
            r2_tiles.append(r2)
            bc = consts.tile([128, T], BF16, tag=f"bc_{jt}", name=f"bc_{jt}")
            nc.scalar.activation(bc, b_in, ACT.Identity, bias=negb)
            bc_tiles.append(bc)

        # -------- BT[t, c, j] = B[j, 128c+t] via PE transposes, fp8 ----------
        bt8 = consts.tile([128, 8, M], FP8)
        for jt in range(2):
            for c in range(8):
                ps_t = ps_aux.tile([128, 128], BF16, tag="aux")
                nc.tensor.transpose(ps_t, bc_tiles[jt][:, 128 * c:128 * (c + 1)], ident_bf)
                nc.scalar.copy(bt8[:, c, jt * 128:(jt + 1) * 128], ps_t)

        # ---------------- main correlation loop ------------------------------
        cmax_p = [
            consts.tile([128, 16, NLOC], F32, tag=f"cmax_{jt}", name=f"cmax_{jt}")
            for jt in range(2)
        ]
        for ic in range(NCHUNK):
            tau = taup.tile([128, NIC, TAU_U], FP8, tag="tau")
            src = apad_d[NIC * ic:NIC * (ic + 1), 0:TAU_U]
            v = src.unsqueeze(0).broadcast_to((128, NIC, TAU_U))
            lst = v.ap
            lst[0] = [1, 128]  # Hankel: dest partition t reads Apad at +t elements
            v.ap = lst
            nc.sync.dma_start(tau[:, :, :], v)
            for jt in range(2):
                for lp in range(8):  # lambda pairs -> one 2-bank psum tile
                    ps = ps_main.tile([128, 2, NIC, 128], F32, tag="grp")
                    for q in range(2):
                        lam = 2 * lp + q
                        # valid double-chunks: e0 = 2dc - lam + 8 in [-1, 8]
                        dcs = [dc for dc in range(4) if -1 <= 2 * dc - lam + 8 <= 8]
                        for k, dc in enumerate(dcs):
                            u0 = 128 * (2 * dc - lam + 9)
                            rhs = tau[:, :, u0:u0 + 256].rearrange(
                                "p r (i d) -> p i r d", i=2
                            )
                            nc.tensor.matmul(
                                ps[:, q],
                                lhsT=bt8[:, 2 * dc:2 * dc + 2, jt * 128:(jt + 1) * 128],
                                rhs=rhs,
                                perf_mode=DROW,
                                start=(k == 0),
                                stop=(k == len(dcs) - 1),
                            )
                    nc.vector.reduce_max(
                        cmax_p[jt][:, 2 * lp:2 * lp + 2, NIC * ic:NIC * (ic + 1)],
                        ps[:, :, :, :],
                        axis=X,
                    )

        # ---------------- normalize + transpose to (i, j) ---------------------
        dist_t = prep.tile([NLOC, M], F32)
        for jt in range(2):
            cm2 = prep.tile([128, NLOC], F32, tag="cm2")
            nc.vector.reduce_max(cm2, cmax_p[jt].rearrange("p l i -> p i l"), axis=X)
            cms = prep.tile([128, NLOC], F32, tag="cms")
            nc.vector.tensor_scalar(cms, cm2, r2_tiles[jt], None, op0=ALU.mult)
            ps_d = ps_aux.tile([NLOC, 128], F32, tag="aux")
            nc.tensor.transpose(ps_d, cms, ident_f32)
            nc.vector.tensor_scalar(dist_t[:, jt * 128:(jt + 1) * 128], ps_d, r1, None, op0=ALU.mult)

        # ---------------- cross-entropy (sum over local rows) -----------------
        mrow = prep.tile([NLOC, 1], F32)
        nc.vector.reduce_max(mrow, dist_t, axis=X)
        negm = prep.tile([NLOC, 1], F32)
        nc.vector.tensor_scalar_mul(negm, mrow, -1.0)
        expj = prep.tile([NLOC, M], F32)
        sumexp = prep.tile([NLOC, 1], F32)
        nc.scalar.activation(expj, dist_t, ACT.Exp, bias=negm, accum_out=sumexp)
        lse = prep.tile([NLOC, 1], F32)
        nc.scalar.activation(lse, sumexp, ACT.Ln)
        onehot = prep.tile([NLOC, M], F32)
        nc.vector.tensor_scalar(onehot, jidx_f, sp_f, None, op0=ALU.is_equal)
        junk_p = prep.tile([NLOC, M], F32)
        picked = prep.tile([NLOC, 1], F32)
        nc.vector.scalar_tensor_tensor(
            junk_p, in0=dist_t, scalar=1.0, in1=onehot, op0=ALU.mult, op1=ALU.mult, accum_out=picked
        )
        term = prep.tile([NLOC, 1], F32)
        nc.vector.tensor_add(term, lse, mrow)
        term2 = prep.tile([NLOC, 1], F32)
        nc.vector.tensor_sub(term2, term, picked)
        ps_l = ps_aux.tile([1, 1], F32, tag="aux")
        nc.tensor.matmul(ps_l, lhsT=term2, rhs=ones_col, start=True, stop=True)
        lsb = prep.tile([1, 1], F32)
        nc.vector.tensor_copy(lsb, ps_l)

        # ---------------- loss AllReduce across the 8 cores -------------------
        ls_in = dram.tile([1, 1], F32, tag="ls_in", name="ls_in")
        nc.gpsimd.dma_start(ls_in[:, :], lsb)
        ls_out = dram.tile([1, 1], F32, tag="ls_out", name="ls_out")
        nc.gpsimd.collective_compute(
            "AllReduce",
            ALU.add,
            replica_groups=GROUPS,
            ins=[ls_in.opt()],
            outs=[ls_out.opt()],
        )
        nc.gpsimd.dma_start(loss_part[:, :], ls_out[:, :])

    nc.finalize()
    return nc


# --------------------------------------------------------------------------
# Host runner: build the jitted shard_map executable ONCE and reuse it.
# --------------------------------------------------------------------------

_RUNNER = None
LAST_RESULT = None


def _build_runner():
    import jax
    from jax.sharding import Mesh, PartitionSpec
    try:
        from jax import shard_map  # jax >= 0.8
    except ImportError:
        from jax.experimental.shard_map import shard_map
    from concourse import bass2jax

    nc = build_nc()
    bass2jax.install_neuronx_cc_hook()
    assert nc.dbg_addr is None

    partition_name = nc.partition_id_tensor.name if nc.partition_id_tensor else None
    in_names, out_names, out_avals, zero_shapes = [], [], [], []
    for alloc in nc.m.functions[0].allocations:
        if not isinstance(alloc, mybir.MemoryLocationSet):
            continue
        name = alloc.memorylocations[0].name
        if alloc.kind == "ExternalInput":
            if name != partition_name:
                in_names.append(name)
        elif alloc.kind == "ExternalOutput":
            out_names.append(name)
            shape = tuple(alloc.tensor_shape)
            dtype = mybir.dt.np(alloc.dtype)
            out_avals.append(jax.core.ShapedArray(shape, dtype))
            zero_shapes.append((shape, dtype))
    n_params = len(in_names)
    n_outs = len(out_avals)
    all_in_names = list(in_names) + list(out_names)
    if partition_name is not None:
        all_in_names.append(partition_name)
    donate = tuple(range(n_params, n_params + n_outs))

    def _body(*args):
        operands = list(args)
        if partition_name is not None:
            operands.append(bass2jax.partition_id_tensor())
        outs = bass2jax._bass_exec_p.bind(
            *operands,
            out_avals=tuple(out_avals),
            in_names=tuple(all_in_names),
            out_names=tuple(out_names),
            lowering_input_output_aliases=(),
            sim_require_finite=True,
            sim_require_nnan=True,
            nc=nc,
        )
        return tuple(outs)

    devices = jax.devices()[:NCORES]
    assert len(devices) == NCORES, f"need {NCORES} devices, have {len(jax.devices())}"
    mesh = Mesh(np.asarray(devices), ("core",))
    in_specs = (PartitionSpec("core"),) * (n_params + n_outs)
    out_specs = (PartitionSpec("core"),) * n_outs
    try:
        smapped = shard_map(
            _body, mesh=mesh, in_specs=in_specs, out_specs=out_specs, check_rep=False
        )
    except TypeError:  # newer jax renamed check_rep
        smapped = shard_map(
            _body, mesh=mesh, in_specs=in_specs, out_specs=out_specs, check_vma=False
        )
    sharded = jax.jit(smapped, donate_argnums=donate, keep_unused=True)
    return {
        "sharded": sharded,
        "in_names": in_names,
        "out_names": out_names,
        "zero_shapes": zero_shapes,
    }


def run(zis, zjs, speeds, trace=False):
    global _RUNNER, LAST_RESULT
    LAST_RESULT = None
    if _RUNNER is None:
        _RUNNER = _build_runner()
    r = _RUNNER

    zq = np.asarray(zis, dtype=np.float32).astype(NP_FP8)       # (256,1024) fp8
    jq = np.asarray(zjs, dtype=np.float32).astype(NP_FP8)       # (256,1024) fp8
    sp = np.asarray(speeds).astype(np.int32).reshape(M, 1)      # (256,1) i32
    by_name = {"zis_loc": zq, "zjs_shard": jq, "speeds_loc": sp}
    concat_in = [np.ascontiguousarray(by_name[n]) for n in r["in_names"]]
    concat_zeros = [
        np.zeros((NCORES * s[0], *s[1:]), d) for (s, d) in r["zero_shapes"]
    ]
    outs = r["sharded"](*concat_in, *concat_zeros)
    # loss is AllReduced on-device: every shard holds the global sum, so we
    # only pull one (1,1) shard through the tunnel.
    loss = np.asarray(outs[0].addressable_shards[0].data)
    return np.float32(loss[0, 0])


def kernel(zis, zjs, speeds):
    return run(zis, zjs, speeds, trace=False)


# revision 5
# speedup vs baseline: 12.8845x; 1.2075x over previous
"""Trainium2 Bass kernel for nn_Cont_InfoNCE (pairwise max cross-correlation + CE loss).

Math: the reference's irfft(F1[i] * conj(F2[j]) / power) is the linear
cross-correlation of the centered rows at every lag, scaled by the positive
constant 1/(power*(T-1)).  max over lags therefore commutes with the scaling,
so dist[i,j] = max_l sum_t f1c[i,t] * f2c[j,t+l] / (1023*s1[i]*s2[j]).

We compute the correlation at all lags as dense fp8e4m3 DoubleRow matmuls on
the tensor engine (fp32 PSUM accumulation; fp8 rounding contributes ~1e-5
relative loss error), max-reduce over lags on the vector engine, and do the
row-wise CE on device.

Sharding + host I/O (latency-optimized for the axon-tunneled cores: the
tunnel has ~75 ms RTT and ~80 MB/s bandwidth, so wire bytes and round
trips dominate; the NEFF itself is sub-ms):
  - zis and zjs ship as fp8e4m3 (quantizing the *inputs* to fp8 moves the
    loss by ~3e-6 relative — measured against the f32 reference — because
    the kernel's matmuls are fp8 anyway and the CE is insensitive).
  - rows of both zis and zjs are sharded across the 8 cores (32 rows
    each); the kernel AllGathers zjs on-device over NeuronLink instead of
    the host shipping 8 replicas through the tunnel.
  - each core's partial CE loss is AllReduced on-device; the host fetches
    a single replicated (1,1) scalar from one shard.
  - the jitted shard_map executable is built once and cached; calling
    bass_utils.run_bass_kernel_spmd per-call would rebuild jax.jit each
    time (~330 ms of retrace/relower per call).

Tiling (per core; A = centered local zis rows (32,1024), B = centered zjs):
  Apad[i]    = [0^255, A[i], 0^257]                       (32, 1536) fp8
  Tau[t,i,u] = Apad[i, u+t]          (Hankel gather via DMA from DRAM)
  BT[t,c,j]  = B[j, 128c+t]          (PE transposes, bf16 -> fp8 on copy-out)
  for lam in 0..15, jt in 0..1, ic in 0..7:
    psum[j,ii,d'] += BT[:, 2dc:2dc+2, jtile].T @ Tau[:, ic, u0:u0+256]  (DoubleRow)
      over dc with u0 = 128*(2dc - lam + 9); pair halves are the two
      128-chunks of t, matching the production [P, ksub, free] convention.
  psum[j,ii,d'] equals C[i, j, l] at lag l = 128*lam - 897 - d', covering
  every lag in [-1024, 1023] exactly once (the l = -1024 slot is identically
  0, mirroring the reference's zero-overlap k=1024 slot).
"""

import sys

if "/opt/trn_rl_repo" not in sys.path:
    sys.path.insert(0, "/opt/trn_rl_repo")

from contextlib import ExitStack

import numpy as np

import concourse.mybir as mybir
from concourse import bacc, tile
from concourse.masks import make_identity

F32 = mybir.dt.float32
BF16 = mybir.dt.bfloat16
FP8 = mybir.dt.float8e4
I32 = mybir.dt.int32
X = mybir.AxisListType.X
ALU = mybir.AluOpType
ACT = mybir.ActivationFunctionType
DROW = mybir.MatmulPerfMode.DoubleRow

M, T = 256, 1024
NCORES = 8
NLOC = M // NCORES  # 32 rows of zis (and of zjs) per core
NIC = 4             # i-rows per i-chunk
NCHUNK = NLOC // NIC  # 8 i-chunks
TAU_U = 1408        # Hankel window extent: covers e0 in [-1, 8], +256 window
APAD = 1536         # 255 zeros + 1024 + 257 zeros
GROUPS = [list(range(NCORES))]

NP_FP8 = mybir.dt.np(FP8)


def _rsqrt_scaled(nc, pool, out, ss, k, parts, tag):
    """out = sqrt(k / ss), elementwise on a (parts,1) fp32 column.

    vector.reciprocal (accurate iterative divide) + ACT Sqrt + one Newton
    step to wash out the Sqrt table's loose ULP budget.
    """
    a = pool.tile([parts, 1], F32, tag=tag + "_a")
    nc.vector.reciprocal(a, ss)
    v = pool.tile([parts, 1], F32, tag=tag + "_v")
    nc.vector.tensor_scalar_mul(v, a, float(k))
    y0 = pool.tile([parts, 1], F32, tag=tag + "_y0")
    nc.scalar.sqrt(y0, v)
    ry = pool.tile([parts, 1], F32, tag=tag + "_ry")
    nc.vector.reciprocal(ry, y0)
    t2 = pool.tile([parts, 1], F32, tag=tag + "_t2")
    # t2 = (v * 0.5) * (1/y0)
    nc.vector.scalar_tensor_tensor(t2, in0=v, scalar=0.5, in1=ry, op0=ALU.mult, op1=ALU.mult)
    # out = (y0 * 0.5) + t2
    nc.vector.scalar_tensor_tensor(out, in0=y0, scalar=0.5, in1=t2, op0=ALU.mult, op1=ALU.add)


def _row_stats(nc, pool, in_tile, parts, tag):
    """Returns (negmean, ss) for each row of in_tile, computed on ScalarE.

    ss = sum((x - mean)^2) = sum(x^2) - T*mean^2; the only DVE use is the
    final tiny (parts,1) combine.
    """
    junk1 = pool.tile([parts, T], BF16, tag=tag + "_j1")
    rsum = pool.tile([parts, 1], F32, tag=tag + "_rsum")
    nc.scalar.activation(junk1, in_tile, ACT.Identity, accum_out=rsum)
    junk2 = pool.tile([parts, T], BF16, tag=tag + "_j2")
    ssraw = pool.tile([parts, 1], F32, tag=tag + "_ssraw")
    nc.scalar.activation(junk2, in_tile, ACT.Square, accum_out=ssraw)
    negmean = pool.tile([parts, 1], F32, tag=tag + "_negmean")
    nc.scalar.mul(negmean, rsum, -1.0 / T)
    mu2 = pool.tile([parts, 1], F32, tag=tag + "_mu2")
    nc.scalar.activation(mu2, negmean, ACT.Square)
    ss = pool.tile([parts, 1], F32, tag=tag + "_ss")
    nc.vector.scalar_tensor_tensor(ss, in0=mu2, scalar=-float(T), in1=ssraw, op0=ALU.mult, op1=ALU.add)
    return negmean, ss


def build_nc():
    nc = bacc.Bacc("TRN2", target_bir_lowering=False, num_devices=NCORES)
    zis_loc = nc.dram_tensor("zis_loc", [NLOC, T], FP8, kind="ExternalInput")
    zjs_shard = nc.dram_tensor("zjs_shard", [NLOC, T], FP8, kind="ExternalInput")
    speeds_loc = nc.dram_tensor("speeds_loc", [NLOC, 1], I32, kind="ExternalInput")
    loss_part = nc.dram_tensor("loss_part", [1, 1], F32, kind="ExternalOutput")

    with tile.TileContext(nc) as tc, ExitStack() as ctx:
        consts = ctx.enter_context(tc.tile_pool(name="consts", bufs=1))
        prep = ctx.enter_context(tc.tile_pool(name="prep", bufs=2))
        dram = ctx.enter_context(tc.tile_pool(name="dram", bufs=1, space="DRAM"))
        taup = ctx.enter_context(tc.tile_pool(name="taup", bufs=3))
        ps_aux = ctx.enter_context(tc.tile_pool(name="ps_aux", bufs=2, space="PSUM"))
        ps_main = ctx.enter_context(tc.tile_pool(name="ps_main", bufs=3, space="PSUM"))

        # ------------- zjs AllGather (fp8 shard -> full 256 rows) ------------
        zj_in = dram.tile([NLOC, T], FP8, tag="zj_in", name="zj_in")
        nc.gpsimd.dma_start(zj_in[:, :], zjs_shard[:, :])
        zj_all = dram.tile([M, T], FP8, tag="zj_all", name="zj_all")
        nc.gpsimd.collective_compute(
            "AllGather",
            mybir.AluOpType.bypass,
            replica_groups=GROUPS,
            ins=[zj_in.opt()],
            outs=[zj_all.opt()],
        )

        # ---------------- constants ----------------
        ident_bf = consts.tile([128, 128], BF16)
        make_identity(nc, ident_bf)
        ident_f32 = consts.tile([128, 128], F32)
        make_identity(nc, ident_f32)
        ones_col = consts.tile([NLOC, 1], F32)
        nc.gpsimd.memset(ones_col, 1.0)
        jidx_i = consts.tile([NLOC, M], I32)
        nc.gpsimd.iota(jidx_i, [[1, M]], base=0, channel_multiplier=0)
        jidx_f = consts.tile([NLOC, M], F32)
        nc.scalar.copy(jidx_f, jidx_i)
        sp_i = prep.tile([NLOC, 1], I32)
        nc.sync.dma_start(sp_i, speeds_loc[:, :])
        sp_f = prep.tile([NLOC, 1], F32)
        nc.scalar.copy(sp_f, sp_i)

        # ---------------- A (local zis rows): stats, center -> fp8 Apad ------
        a_in = prep.tile([NLOC, T], FP8)
        nc.sync.dma_start(a_in, zis_loc[:, :])
        nega, ss1 = _row_stats(nc, prep, a_in, NLOC, "a")
        r1 = prep.tile([NLOC, 1], F32)
        _rsqrt_scaled(nc, prep, r1, ss1, 1.0 / (T - 1), NLOC, "r1")  # 1/((T-1)*s1)

        apad_sb = prep.tile([NLOC, APAD], FP8)
        nc.gpsimd.memset(apad_sb, 0.0)
        nc.scalar.activation(apad_sb[:, 255:255 + T], a_in, ACT.Identity, bias=nega)
        apad_d = dram.tile([NLOC, APAD], FP8, tag="apad", name="apad")
        nc.sync.dma_start(apad_d[:, :], apad_sb[:, :])

        # ---------------- B (all zjs rows): stats, center -> bf16 ------------
        bc_tiles = []
        r2_tiles = []
        for jt in range(2):
            b_in = prep.tile([128, T], FP8, tag="b_in")
            nc.sync.dma_start(b_in, zj_all[jt * 128:(jt + 1) * 128, :])
            negb, ss2 = _row_stats(nc, prep, b_in, 128, "b")
            r2 = consts.tile([128, 1], F32, tag=f"r2_{jt}", name=f"r2_{jt}")
            _rsqrt_scaled(nc, prep, r2, ss2, float(T - 1), 128, "r2")  # 1/s2
            r2_tiles.append(r2)
            bc = consts.tile([128, T], BF16, tag=f"bc_{jt}", name=f"bc_{jt}")
            nc.scalar.activation(bc, b_in, ACT.Identity, bias=negb)
            bc_tiles.append(bc)

        # -------- BT[t, c, j] = B[j, 128c+t] via PE transposes, fp8 ----------
        bt8 = consts.tile([128, 8, M], FP8)
        for jt in range(2):
            for c in range(8):
                ps_t = ps_aux.tile([128, 128], BF16, tag="aux")
                nc.tensor.transpose(ps_t, bc_tiles[jt][:, 128 * c:128 * (c + 1)], ident_bf)
                nc.scalar.copy(bt8[:, c, jt * 128:(jt + 1) * 128], ps_t)

        # ---------------- main correlation loop ------------------------------
        cmax_p = [
            consts.tile([128, 16, NLOC], F32, tag=f"cmax_{jt}", name=f"cmax_{jt}")
            for jt in range(2)
        ]
        for ic in range(NCHUNK):
            tau = taup.tile([128, NIC, TAU_U], FP8, tag="tau")
            src = apad_d[NIC * ic:NIC * (ic + 1), 0:TAU_U]
            v = src.unsqueeze(0).broadcast_to((128, NIC, TAU_U))
            lst = v.ap
            lst[0] = [1, 128]  # Hankel: dest partition t reads Apad at +t elements
            v.ap = lst
            nc.sync.dma_start(tau[:, :, :], v)
            for jt in range(2):
                for lp in range(8):  # lambda pairs -> one 2-bank psum tile
                    ps = ps_main.tile([128, 2, NIC, 128], F32, tag="grp")
                    for q in range(2):
                        lam = 2 * lp + q
                        # valid double-chunks: e0 = 2dc - lam + 8 in [-1, 8]
                        dcs = [dc for dc in range(4) if -1 <= 2 * dc - lam + 8 <= 8]
                        for k, dc in enumerate(dcs):
                            u0 = 128 * (2 * dc - lam + 9)
                            rhs = tau[:, :, u0:u0 + 256].rearrange(
                                "p r (i d) -> p i r d", i=2
                            )
                            nc.tensor.matmul(
                                ps[:, q],
                                lhsT=bt8[:, 2 * dc:2 * dc + 2, jt * 128:(jt + 1) * 128],
                                rhs=rhs,
                                perf_mode=DROW,
                                start=(k == 0),
                                stop=(k == len(dcs) - 1),
                            )
                    nc.vector.reduce_max(
                        cmax_p[jt][:, 2 * lp:2 * lp + 2, NIC * ic:NIC * (ic + 1)],
                        ps[:, :, :, :],
                        axis=X,
                    )

        # ---------------- normalize + transpose to (i, j) ---------------------
        dist_t = prep.tile([NLOC, M], F32)
        for jt in range(2):
            cm2 = prep.tile([128, NLOC], F32, tag="cm2")
            nc.vector.reduce_max(cm2, cmax_p[jt].rearrange("p l i -> p i l"), axis=X)
            cms = prep.tile([128, NLOC], F32, tag="cms")
            nc.vector.tensor_scalar(cms, cm2, r2_tiles[jt], None, op0=ALU.mult)
            ps_d = ps_aux.tile([NLOC, 128], F32, tag="aux")
            nc.tensor.transpose(ps_d, cms, ident_f32)
            nc.vector.tensor_scalar(dist_t[:, jt * 128:(jt + 1) * 128], ps_d, r1, None, op0=ALU.mult)

        # ---------------- cross-entropy (sum over local rows) -----------------
        mrow = prep.tile([NLOC, 1], F32)
        nc.vector.reduce_max(mrow, dist_t, axis=X)
        negm = prep.tile([NLOC, 1], F32)
        nc.vector.tensor_scalar_mul(negm, mrow, -1.0)
        expj = prep.tile([NLOC, M], F32)
        sumexp = prep.tile([NLOC, 1], F32)
        nc.scalar.activation(expj, dist_t, ACT.Exp, bias=negm, accum_out=sumexp)
        lse = prep.tile([NLOC, 1], F32)
        nc.scalar.activation(lse, sumexp, ACT.Ln)
        onehot = prep.tile([NLOC, M], F32)
        nc.vector.tensor_scalar(onehot, jidx_f, sp_f, None, op0=ALU.is_equal)
        junk_p = prep.tile([NLOC, M], F32)
        picked = prep.tile([NLOC, 1], F32)
        nc.vector.scalar_tensor_tensor(
            junk_p, in0=dist_t, scalar=1.0, in1=onehot, op0=ALU.mult, op1=ALU.mult, accum_out=picked
        )
        term = prep.tile([NLOC, 1], F32)
        nc.vector.tensor_add(term, lse, mrow)
        term2 = prep.tile([NLOC, 1], F32)
        nc.vector.tensor_sub(term2, term, picked)
        ps_l = ps_aux.tile([1, 1], F32, tag="aux")
        nc.tensor.matmul(ps_l, lhsT=term2, rhs=ones_col, start=True, stop=True)
        lsb = prep.tile([1, 1], F32)
        nc.vector.tensor_copy(lsb, ps_l)

        # ---------------- loss AllReduce across the 8 cores -------------------
        ls_in = dram.tile([1, 1], F32, tag="ls_in", name="ls_in")
        nc.gpsimd.dma_start(ls_in[:, :], lsb)
        ls_out = dram.tile([1, 1], F32, tag="ls_out", name="ls_out")
        nc.gpsimd.collective_compute(
            "AllReduce",
            ALU.add,
            replica_groups=GROUPS,
            ins=[ls_in.opt()],
            outs=[ls_out.opt()],
        )
        nc.gpsimd.dma_start(loss_part[:, :], ls_out[:, :])

    nc.finalize()
    return nc


# --------------------------------------------------------------------------
# Host runner: build the jitted shard_map executable ONCE and reuse it.
# --------------------------------------------------------------------------

_RUNNER = None
LAST_RESULT = None


def _build_runner():
    import jax
    from jax.sharding import Mesh, PartitionSpec
    try:
        from jax import shard_map  # jax >= 0.8
    except ImportError:
        from jax.experimental.shard_map import shard_map
    from concourse import bass2jax

    nc = build_nc()
    bass2jax.install_neuronx_cc_hook()
    assert nc.dbg_addr is None

    partition_name = nc.partition_id_tensor.name if nc.partition_id_tensor else None
    in_names, out_names, out_avals, zero_shapes = [], [], [], []
    for alloc in nc.m.functions[0].allocations:
        if not isinstance(alloc, mybir.MemoryLocationSet):
            continue
        name = alloc.memorylocations[0].name
        if alloc.kind == "ExternalInput":
            if name != partition_name:
                in_names.append(name)
        elif alloc.kind == "ExternalOutput":
            out_names.append(name)
            shape = tuple(alloc.tensor_shape)
            dtype = mybir.dt.np(alloc.dtype)
            out_avals.append(jax.core.ShapedArray(shape, dtype))
            zero_shapes.append((shape, dtype))
    n_params = len(in_names)
    n_outs = len(out_avals)
    all_in_names = list(in_names) + list(out_names)
    if partition_name is not None:
        all_in_names.append(partition_name)
    donate = tuple(range(n_params, n_params + n_outs))

    def _body(*args):
        operands = list(args)
        if partition_name is not None:
            operands.append(bass2jax.partition_id_tensor())
        outs = bass2jax._bass_exec_p.bind(
            *operands,
            out_avals=tuple(out_avals),
            in_names=tuple(all_in_names),
            out_names=tuple(out_names),
            lowering_input_output_aliases=(),
            sim_require_finite=True,
            sim_require_nnan=True,
            nc=nc,
        )
        return tuple(outs)

    devices = jax.devices()[:NCORES]
    assert len(devices) == NCORES, f"need {NCORES} devices, have {len(jax.devices())}"
    mesh = Mesh(np.asarray(devices), ("core",))
    in_specs = (PartitionSpec("core"),) * (n_params + n_outs)
    out_specs = (PartitionSpec("core"),) * n_outs
    try:
        smapped = shard_map(
            _body, mesh=mesh, in_specs=in_specs, out_specs=out_specs, check_rep=False
        )
    except TypeError:  # newer jax renamed check_rep
        smapped = shard_map(
            _body, mesh=mesh, in_specs=in_specs, out_specs=out_specs, check_vma=False
        )
    sharded = jax.jit(smapped, donate_argnums=donate, keep_unused=True)

    # f32 -> fp8 wire cast, jitted on the CPU backend: ~0.4 ms/MB vs ~1.8 ms
    # for ml_dtypes astype, bit-identical output.
    import jax.numpy as jnp

    try:
        _cast = jax.jit(lambda a: a.astype(jnp.float8_e4m3), backend="cpu")
        _cast(np.zeros((2, 2), np.float32))  # smoke-test the backend kwarg

        def fp8_cast(a):
            return np.asarray(_cast(a))
    except Exception:
        def fp8_cast(a):
            return a.astype(NP_FP8)

    zeros = [np.zeros((NCORES * s[0], *s[1:]), d) for (s, d) in zero_shapes]
    return {
        "sharded": sharded,
        "in_names": in_names,
        "out_names": out_names,
        "zero_shapes": zero_shapes,
        "zeros": zeros,
        "fp8_cast": fp8_cast,
    }


def run(zis, zjs, speeds, trace=False):
    global _RUNNER, LAST_RESULT
    LAST_RESULT = None
    if _RUNNER is None:
        _RUNNER = _build_runner()
    r = _RUNNER

    cast = r["fp8_cast"]
    zq = cast(np.ascontiguousarray(zis, dtype=np.float32))      # (256,1024) fp8
    jq = cast(np.ascontiguousarray(zjs, dtype=np.float32))      # (256,1024) fp8
    sp = np.asarray(speeds).astype(np.int32).reshape(M, 1)      # (256,1) i32
    by_name = {"zis_loc": zq, "zjs_shard": jq, "speeds_loc": sp}
    concat_in = [by_name[n] for n in r["in_names"]]
    outs = r["sharded"](*concat_in, *r["zeros"])
    # loss is AllReduced on-device: every shard holds the global sum, so we
    # only pull one (1,1) shard through the tunnel.
    loss = np.asarray(outs[0].addressable_shards[0].data)
    return np.float32(loss[0, 0])


def kernel(zis, zjs, speeds):
    return run(zis, zjs, speeds, trace=False)
